# revision 20
# baseline (speedup 1.0000x reference)
"""Trainium2 Bass kernel for nn_CMAModel (memory-augmented causal attention).

Sharding: 8 cores = 2 batches x 4 head-groups. Each core handles one batch and
4 heads (256 channels); the output projection is row-parallel and the 4
per-batch partials are summed on the host (bf16 partials).

Per-core device program (channels on partitions, bf16 matmul operands):
  proj: qT/kT (paired PSUM banks), V rows (with ones col for row-sums),
        gate logits -> tanh (same ACT table set as Exp; sigmoid = .5*tanh+.5)
  attention per (head-pair mq, T-chunk j): for each 128-row S-tile,
        scoresT for heads A,B as two K=64 row-tiled matmuls into a 2-bank
        PSUM pair -> ONE Exp activation over both banks -> causal tri-mask
        on diagonal tiles -> PV matmuls accumulate Ac/Am [65,512] per head
        (ones col gives row-sums Z for free)
  combine: Y = (Ac + g*Am)/Z. g and 1/Z are partition-broadcast with rank-1
        matmuls on the PE (lhsT=indicator const, rhs=rows of gzt), so no
        DRAM round trips. Z-recip runs 128-wide via small SBUF reshape DMAs.
  conv + out-proj interleaved per T-chunk: depthwise causal conv K=4 +
        residual + bias, then out partial [512,1024] -> DRAM (bf16).
"""
import contextlib
import ctypes
import os
import sys
import types

import numpy as np

# ---------------------------------------------------------------- constants
B, T, C = 2, 2048, 1024
H, HD = 16, 64
M = 256
G = 4                 # head-groups (cores per batch)
HPG = H // G          # 4 heads per core
CPG = HPG * HD        # 256 channels per core
S = T + 2 * M         # 2560 kv rows
SM = 2 * M            # 512 memory rows
NKT = C // 128        # 8 contraction tiles
NST = S // 128        # 20 S tiles (16 chunk + 4 mem)
TC = 512              # T chunk size
NTC = T // TC         # 4
K = 4                 # conv taps
SCALE = 1.0 / float(np.sqrt(HD))

_BUILT = None


# ------------------------------------------------------- axon NTFF hook shim
def _install_ntff_hook():
    """The agent image lacks antenv.axon_hooks; synthesize it so
    run_bass_kernel_spmd(trace=True) can capture NTFF profiles."""
    if "antenv.axon_hooks" in sys.modules:
        return
    so_path = "/opt/axon/libaxon_pjrt.so"
    hook = None
    if os.path.exists(so_path):
        try:
            lib = ctypes.CDLL(so_path)
            if hasattr(lib, "axon_start_nrt_profile"):
                lib.axon_start_nrt_profile.argtypes = [
                    ctypes.POINTER(ctypes.c_int64),
                    ctypes.c_size_t,
                ]
                lib.axon_start_nrt_profile.restype = ctypes.c_int64
                lib.axon_stop_nrt_profile.argtypes = [ctypes.c_char_p]
                lib.axon_stop_nrt_profile.restype = ctypes.c_int64

                @contextlib.contextmanager
                def _hook(output_dir, device_ids):
                    import jax

                    jax.devices()
                    if device_ids:
                        ids = (ctypes.c_int64 * len(device_ids))(*device_ids)
                        rc = lib.axon_start_nrt_profile(ids, len(device_ids))
                    else:
                        rc = lib.axon_start_nrt_profile(None, 0)
                    if rc != 0:
                        raise RuntimeError(f"axon_start_nrt_profile rc={rc}")
                    try:
                        yield
                    finally:
                        n = lib.axon_stop_nrt_profile(str(output_dir).encode())
                        if n < 0:
                            raise RuntimeError(f"axon_stop_nrt_profile rc={n}")

                hook = _hook
        except OSError:
            pass
    mod = types.ModuleType("antenv.axon_hooks")
    mod.get_axon_ntff_profile_hook = lambda: hook
    mod.set_axon_ntff_profile_hook = lambda h: None
    sys.modules["antenv.axon_hooks"] = mod


# ------------------------------------------------------------- device build
def _build_program():
    import concourse.tile as tile
    from concourse import bacc, mybir
    from concourse.masks import make_upper_triangular

    f32 = mybir.dt.float32
    mdt = mybir.dt.bfloat16

    nc = bacc.Bacc("TRN2", target_bir_lowering=False, debug=False, num_devices=8)

    xT = nc.dram_tensor("xT", [C, T], mdt, kind="ExternalInput").ap()
    memT = nc.dram_tensor("memT", [C, SM], mdt, kind="ExternalInput").ap()
    WqT = nc.dram_tensor("WqT", [C, CPG], mdt, kind="ExternalInput").ap()
    WkT = nc.dram_tensor("WkT", [C, CPG], mdt, kind="ExternalInput").ap()
    WvTa = nc.dram_tensor("WvTa", [C, 64 * HPG], mdt, kind="ExternalInput").ap()
    WgT = nc.dram_tensor("WgT", [C, HPG], mdt, kind="ExternalInput").ap()
    gb2 = nc.dram_tensor("gb2", [HPG, 1], f32, kind="ExternalInput").ap()
    WoT = nc.dram_tensor("WoT", [CPG, C], mdt, kind="ExternalInput").ap()
    cw = nc.dram_tensor("cw", [CPG, K], f32, kind="ExternalInput").ap()
    cb = nc.dram_tensor("cb", [CPG, 1], f32, kind="ExternalInput").ap()
    out = nc.dram_tensor("out", [T, C], mdt, kind="ExternalOutput").ap()

    Exp = mybir.ActivationFunctionType.Exp
    Tanh = mybir.ActivationFunctionType.Tanh

    with tile.TileContext(nc) as tc:
        with contextlib.ExitStack() as ctx:
            const = ctx.enter_context(tc.tile_pool(name="const", bufs=1))
            xpool = ctx.enter_context(tc.tile_pool(name="xpool", bufs=2))
            sb = ctx.enter_context(tc.tile_pool(name="sb", bufs=1))
            work = ctx.enter_context(tc.tile_pool(name="work", bufs=4))
            small = ctx.enter_context(tc.tile_pool(name="small", bufs=1))
            psum = ctx.enter_context(
                tc.tile_pool(name="psum", bufs=1, space="PSUM")
            )

            # ---- constants / weights. Order matters: the first proj
            # matmul needs only wq/wk + the first x k-tiles, so issue those
            # DMAs first and defer the rest.
            wq_s = const.tile([128, NKT, CPG], mdt)
            nc.sync.dma_start(out=wq_s, in_=WqT.rearrange("(a p) n -> p a n", p=128))
            xTr0 = xT.rearrange("(a p) t -> p a t", p=128)
            xh0 = xpool.tile([128, NKT, T // 2], mdt, tag="xbig", name="xh0")
            nc.sync.dma_start(out=xh0[:, 0, :], in_=xTr0[:, 0, :T // 2])
            wk_s = const.tile([128, NKT, CPG], mdt)
            nc.sync.dma_start(out=wk_s, in_=WkT.rearrange("(a p) n -> p a n", p=128))
            for k in range(1, NKT):
                nc.sync.dma_start(out=xh0[:, k, :], in_=xTr0[:, k, :T // 2])
            wva_s = const.tile([128, NKT, 64 * HPG], mdt)
            nc.sync.dma_start(out=wva_s, in_=WvTa.rearrange("(a p) n -> p a n", p=128))
            wg_s = const.tile([128, NKT, HPG], mdt)
            nc.sync.dma_start(out=wg_s, in_=WgT.rearrange("(a p) n -> p a n", p=128))
            mems = xpool.tile([128, NKT, SM], mdt, tag="xmem", name="xmem")
            nc.sync.dma_start(out=mems, in_=memT.rearrange("(a p) t -> p a t", p=128))
            wo_s = const.tile([128, 2, C], mdt)
            nc.sync.dma_start(out=wo_s, in_=WoT.rearrange("(a p) n -> p a n", p=128))
            cw_s = const.tile([128, 2, K], f32)
            nc.sync.dma_start(out=cw_s, in_=cw.rearrange("(a p) n -> p a n", p=128))
            cb_s = const.tile([128, 2, 1], f32)
            nc.sync.dma_start(out=cb_s, in_=cb.rearrange("(a p) n -> p a n", p=128))
            gb2_s = const.tile([HPG, 1], f32)
            nc.sync.dma_start(out=gb2_s, in_=gb2)

            trif = const.tile([128, 128], f32)
            make_upper_triangular(nc, trif, val=1.0, diag=True)
            tri2 = const.tile([128, 2, 128], mdt)
            nc.vector.tensor_copy(tri2[:, 0, :], trif)
            nc.vector.tensor_copy(tri2[:, 1, :], trif)

            # ---- persistent activations
            # qkT_s[:, m, 0, t] = qT, [:, m, 1, t] = kT  (m = channel half)
            qkT_s = sb.tile([128, 2, 2, T], mdt)
            kTm_s = sb.tile([128, 2, SM], mdt)
            V_s = sb.tile([128, NST, 128 * HPG], mdt)
            # broadcast sources all live on partition 0 (engine ops need
            # 32-aligned partition starts): slot 4*mq+{0,1,2,3} =
            # {sigmoid_A, sigmoid_B, recipZ_A, recipZ_B}
            gzt = sb.tile([1, 8, T], mdt)
            # attnout[:, mq, 0, t] = Y, [:, mq, 1, t] = conv result
            attnout = sb.tile([128, 2, 2, T], mdt)

            # one-time inits: ones col in V, gzt ones + recip rows (rows 0-1 /
            # 64-65 are overwritten by the gate tanh per chunk; engine ops
            # need 32-aligned partition starts so memset the whole block)
            oc = V_s[:, :, 64:128 * HPG:128]
            nc.vector.memset(oc, 1.0)

            xTr = xT.rearrange("(a p) t -> p a t", p=128)

            def proj_chunk(xh, tglob, tloc, on_act):
                """q/k/V/gate projections for T columns [tglob, tglob+512).
                on_act: route PSUM->SBUF copies to ScalarE (idle early) or
                VectorE (when ScalarE is busy with attention exps)."""
                cp = nc.scalar.copy if on_act else nc.vector.tensor_copy
                for m in range(2):
                    for w, ws in ((0, wq_s), (1, wk_s)):
                        qk = psum.tile([128, TC], f32, tag="pp", bufs=2,
                                       name=f"qk{tglob}_{m}_{w}")
                        for k in range(NKT):
                            nc.tensor.matmul(
                                qk,
                                ws[:, k, m * 128:(m + 1) * 128],
                                xh[:, k, tloc:tloc + TC],
                                start=(k == 0),
                                stop=(k == NKT - 1),
                            )
                        cp(qkT_s[:, m, w, tglob:tglob + TC], qk)
                for mt in range(TC // 128):
                    st = tglob // 128 + mt
                    pv = psum.tile([128, 64 * HPG], f32, tag="pa", bufs=2,
                                   name=f"pv{st}")
                    for k in range(NKT):
                        nc.tensor.matmul(
                            pv,
                            xh[:, k, tloc + mt * 128:tloc + (mt + 1) * 128],
                            wva_s[:, k, :],
                            start=(k == 0),
                            stop=(k == NKT - 1),
                        )
                    # copy the 4 x 64 v-blocks, skipping the ones columns
                    cp(
                        V_s[:, st, :].rearrange("p (h c) -> p h c", c=128)[:, :, 0:64],
                        pv.rearrange("p (h c) -> p h c", c=64),
                    )
                pg = psum.tile([HPG, TC], f32, tag="pa", bufs=2,
                               name=f"pg{tglob}")
                for k in range(NKT):
                    nc.tensor.matmul(
                        pg,
                        wg_s[:, k, :],
                        xh[:, k, tloc:tloc + TC],
                        start=(k == 0),
                        stop=(k == NKT - 1),
                    )
                # sigmoid(l+b) = .5*tanh((l+b)/2) + .5; the .5 affine folds
                # into the gbc broadcast matmul via the ind/ones-row coeffs
                gtmp = small.tile([HPG, TC], mdt, tag="gt", bufs=2,
                                  name=f"gt{tglob}")
                nc.scalar.activation(
                    gtmp, pg, Tanh, bias=gb2_s, scale=0.5,
                )
                nc.vector.tensor_scalar(
                    gtmp, gtmp, 0.5, 0.5,
                    mybir.AluOpType.mult, mybir.AluOpType.add,
                )
                nc.sync.dma_start(
                    out=gzt[0:1, 0:2, tglob:tglob + TC], in_=gtmp[0:2, :]
                )
                nc.sync.dma_start(
                    out=gzt[0:1, 4:6, tglob:tglob + TC], in_=gtmp[2:4, :]
                )

            def proj_mem(mems):
                mk = psum.tile([128, 2, SM], f32, tag="pa", bufs=2, name="mk")
                for m in range(2):
                    for k in range(NKT):
                        nc.tensor.matmul(
                            mk[:, m, :],
                            wk_s[:, k, m * 128:(m + 1) * 128],
                            mems[:, k, :],
                            start=(k == 0),
                            stop=(k == NKT - 1),
                        )
                nc.scalar.copy(kTm_s, mk)
                for mt in range(SM // 128):
                    st = 16 + mt
                    pv = psum.tile([128, 64 * HPG], f32, tag="pa", bufs=2,
                                   name=f"pvm{mt}")
                    for k in range(NKT):
                        nc.tensor.matmul(
                            pv,
                            mems[:, k, mt * 128:(mt + 1) * 128],
                            wva_s[:, k, :],
                            start=(k == 0),
                            stop=(k == NKT - 1),
                        )
                    nc.scalar.copy(
                        V_s[:, st, :].rearrange("p (h c) -> p h c", c=128)[:, :, 0:64],
                        pv.rearrange("p (h c) -> p h c", c=64),
                    )

            def attn_block(mq, j):
                """Attention accumulation for head pair (2mq, 2mq+1), chunk j.
                Emits everything up to uY = Ac + g*Am (which frees the PSUM
                accumulators without waiting for the Z-reciprocal chain) and
                returns a finish() closure — the reciprocal-dependent final
                multiply — to be emitted after the NEXT block's matmuls so
                the in-order PE queue never stalls on the Z chain."""
                sl = 4 * mq
                hA, hB = 2 * mq, 2 * mq + 1
                nct = 4 * (j + 1)
                js = TC * j
                # gate broadcast on GpSimd (idle engine, off the PE queue)
                gbS = small.tile([64, 2, TC], mdt, tag="gbS", bufs=3,
                                 name=f"gb{mq}_{j}")
                for hb in range(2):
                    nc.gpsimd.partition_broadcast(
                        gbS[:, hb, :], gzt[0:1, sl + hb, js:js + TC]
                    )
                AcAm_A = psum.tile([128, 2, TC], f32, tag="pa", bufs=2,
                                   name=f"aa{mq}_{j}")
                AcAm_B = psum.tile([128, 2, TC], f32, tag="pa", bufs=2,
                                   name=f"ab{mq}_{j}")
                for i in range(nct + 4):
                    is_mem = i >= nct
                    si = (16 + i - nct) if is_mem else i
                    off = 0
                    if not is_mem and si >= 4 * j:
                        off = 128 * si - TC * j
                    sp = psum.tile([128, 2, TC], f32, tag="pp", bufs=2,
                                   name=f"sp{mq}_{j}_{i}")
                    for b, ro in ((0, 0), (1, 64)):
                        kt = (
                            qkT_s[ro:ro + 64, mq, 1, si * 128:(si + 1) * 128]
                            if si < 16
                            else kTm_s[ro:ro + 64, mq,
                                       (si - 16) * 128:(si - 15) * 128]
                        )
                        nc.tensor.matmul(
                            sp[:, b, off:],
                            kt,
                            qkT_s[ro:ro + 64, mq, 0, js + off:js + TC],
                            start=True,
                            stop=True,
                        )
                    Pt = work.tile([128, 2, TC], mdt, tag="P", bufs=6)
                    nc.scalar.activation(
                        Pt[:, :, off:], sp[:, :, off:], Exp, scale=SCALE
                    )
                    if not is_mem and si >= 4 * j:
                        nc.vector.tensor_mul(
                            Pt[:, :, off:off + 128], Pt[:, :, off:off + 128],
                            tri2,
                        )
                    cm = 1 if is_mem else 0
                    first = (i == 0) or (i == nct)
                    last = (i == nct - 1) or (i == nct + 3)
                    nc.tensor.matmul(
                        AcAm_A[:, cm, off:],
                        V_s[:, si, 128 * hA:128 * hA + 128],
                        Pt[:, 0, off:],
                        start=first,
                        stop=last,
                    )
                    nc.tensor.matmul(
                        AcAm_B[:, cm, off:],
                        V_s[:, si, 128 * hB:128 * hB + 128],
                        Pt[:, 1, off:],
                        start=first,
                        stop=last,
                    )
                # Z rows -> 128-wide reciprocal -> gzt recip rows (TT may
                # read at most one PSUM operand, so copy then add)
                zt = small.tile([128, 2, TC], f32, tag="zt", bufs=2,
                                name=f"zt{mq}_{j}")
                uYs = []
                for b, AcAm in ((0, AcAm_A), (1, AcAm_B)):
                    nc.vector.tensor_copy(zt[64:65, b, :], AcAm[64:65, 0, :])
                    nc.vector.tensor_add(
                        zt[64:65, b, :], zt[64:65, b, :], AcAm[64:65, 1, :]
                    )
                    # uY = Ac + g*Am consumes the accumulators now
                    uY = small.tile([64, TC], mdt, tag="uY", bufs=5,
                                    name=f"uY{mq}_{j}_{b}")
                    nc.vector.tensor_mul(uY, AcAm[0:64, 1, :], gbS[:, b, :])
                    nc.vector.tensor_add(uY, uY, AcAm[0:64, 0, :])
                    uYs.append(uY)
                zrf = small.tile([128, 8], f32, tag="zrf", bufs=2,
                                 name=f"zrf{mq}_{j}")
                nc.sync.dma_start(out=zrf, in_=zt[64:65, :, :])
                zrg = small.tile([128, 8], f32, tag="zrg", bufs=2,
                                 name=f"zrg{mq}_{j}")
                nc.vector.reciprocal(zrg, zrf)
                zrb = small.tile([128, 8], mdt, tag="zrb", bufs=2,
                                 name=f"zrb{mq}_{j}")
                nc.vector.tensor_copy(zrb, zrg)
                nc.sync.dma_start(
                    out=gzt[0:1, sl + 2, js:js + TC], in_=zrb[0:64, :]
                )
                nc.sync.dma_start(
                    out=gzt[0:1, sl + 3, js:js + TC], in_=zrb[64:128, :]
                )

                rbS = small.tile([64, 2, TC], mdt, tag="rbS", bufs=3,
                                 name=f"rb{mq}_{j}")
                for hb in range(2):
                    nc.gpsimd.partition_broadcast(
                        rbS[:, hb, :], gzt[0:1, sl + 2 + hb, js:js + TC]
                    )

                def finish():
                    nc.vector.tensor_mul(
                        attnout[0:64, mq, 0, js:js + TC], uYs[0], rbS[:, 0, :]
                    )
                    ybt = small.tile([64, TC], mdt, tag="ybt", bufs=2,
                                     name=f"yb{mq}_{j}")
                    nc.vector.tensor_mul(ybt, uYs[1], rbS[:, 1, :])
                    nc.sync.dma_start(
                        out=attnout[64:128, mq, 0, js:js + TC], in_=ybt
                    )

                return finish

            def conv_chunk(j, mq):
                """depthwise causal conv + residual + bias on chunk j."""
                eng = nc.vector
                js, je = TC * j, TC * (j + 1)
                y = attnout[:, mq, 0, :]
                R = attnout[:, mq, 1, :]
                eng.tensor_scalar_add(
                    R[:, js:je], y[:, js:je], cb_s[:, mq, :]
                )
                ctmp = small.tile([128, TC], mdt, tag=f"ctmp{mq}", bufs=2,
                                  name=f"ct{j}_{mq}")
                for k in range(K):
                    sh = K - 1 - k
                    if sh == 0:
                        eng.tensor_scalar_mul(
                            ctmp, y[:, js:je], cw_s[:, mq, k:k + 1]
                        )
                        eng.tensor_add(R[:, js:je], R[:, js:je], ctmp)
                    else:
                        a = sh if j == 0 else 0
                        eng.tensor_scalar_mul(
                            ctmp[:, a:], y[:, js + a - sh:je - sh],
                            cw_s[:, mq, k:k + 1],
                        )
                        eng.tensor_add(
                            R[:, js + a:je], R[:, js + a:je], ctmp[:, a:]
                        )

            def outproj_chunk(j, mts=None):
                for mt in (range(TC // 128) if mts is None else mts):
                    row = j * 4 + mt
                    po = psum.tile([128, 2, TC], f32, tag="pp", bufs=2,
                                   name=f"po{row}")
                    for nb in range(2):
                        for p in range(2):
                            nc.tensor.matmul(
                                po[:, nb, :],
                                attnout[:, p, 1, row * 128:(row + 1) * 128],
                                wo_s[:, p, nb * TC:(nb + 1) * TC],
                                start=(p == 0),
                                stop=(p == 1),
                            )
                    ot = small.tile([128, 2, TC], mdt, tag="ot", bufs=3,
                                    name=f"ot{row}")
                    nc.vector.tensor_copy(ot, po)
                    nc.sync.dma_start(
                        out=out[row * 128:(row + 1) * 128, :].rearrange(
                            "p (a n) -> p a n", a=2
                        ),
                        in_=ot,
                    )

            # ---- emission: proj c0, mem, c1, then attn j interleaved with
            # remaining proj chunks so PE always has dense work and ACT/DVE
            # overlap.
            proj_chunk(xh0, 0, 0, on_act=True)
            proj_mem(mems)

            xh1 = xpool.tile([128, NKT, T // 2], mdt, tag="xbig", name="xh1")
            for k in range(NKT):
                nc.sync.dma_start(out=xh1[:, k, :], in_=xTr[:, k, T // 2:])

            # Pipelined emission: each block's reciprocal-dependent finish()
            # lands after the next block's matmul burst; conv one slot later;
            # outproj one more. Keeps the in-order PE queue stall-free.
            pending = []

            def drain(now):
                pending.sort(key=lambda e: e[0])
                while pending and pending[0][0] <= now:
                    pending.pop(0)[1]()

            slot = 0
            for j in range(NTC):
                for mq in range(2):
                    fin = attn_block(mq, j)
                    drain(slot)
                    pending.append((slot + 1, fin))
                    pending.append(
                        (slot + 2, lambda jc=j, mqc=mq: conv_chunk(jc, mqc))
                    )
                    slot += 1
                if j < NTC - 1:
                    tg = (j + 1) * TC
                    if tg < T // 2:
                        proj_chunk(xh0, tg, tg, on_act=True)
                    else:
                        proj_chunk(xh1, tg, tg - T // 2, on_act=False)
                    drain(slot)
                    slot += 1
                    pending.append((slot + 1, lambda jc=j: outproj_chunk(jc)))
                else:
                    pending.append((slot + 2, lambda jc=j: outproj_chunk(jc)))
            drain(slot + 4)

    nc.compile()
    return nc


def _get_program():
    global _BUILT
    if _BUILT is None:
        _install_ntff_hook()
        _BUILT = _build_program()
    return _BUILT


# --------------------------------------------------------------- host side
def _b16(a):
    import ml_dtypes

    return np.ascontiguousarray(a, np.float32).astype(ml_dtypes.bfloat16)


def host_prep(inputs):
    x = np.ascontiguousarray(np.asarray(inputs["x"], np.float32))
    fwd = np.asarray(inputs["fwd_mem"], np.float32)
    rev = np.asarray(inputs["rev_mem"], np.float32)
    Wq = np.asarray(inputs["Wq"], np.float32)
    Wk = np.asarray(inputs["Wk"], np.float32)
    Wv = np.asarray(inputs["Wv"], np.float32)
    Wo = np.asarray(inputs["Wo"], np.float32)
    gate_w = np.asarray(inputs["gate_w"], np.float32)
    gate_b = np.asarray(inputs["gate_b"], np.float32)
    canon_w = np.asarray(inputs["canon_w"], np.float32)
    canon_bias = np.asarray(inputs["canon_bias"], np.float32)

    Wg = (gate_w.astype(np.float64) @ Wq.astype(np.float64)).astype(np.float32)

    per_b, per_g = [], []
    for b in range(B):
        per_b.append({
            "xT": _b16(x[b].T),
            "memT": _b16(np.concatenate([fwd[b], rev[b]], axis=0).T),
        })
    for g in range(G):
        cs = slice(g * CPG, (g + 1) * CPG)
        WvTa = np.ascontiguousarray(Wv[cs].T)
        hs = slice(g * HPG, (g + 1) * HPG)
        per_g.append({
            "WqT": _b16(Wq[cs].T),
            "WkT": _b16(Wk[cs].T),
            "WvTa": _b16(WvTa),
            "WgT": _b16(Wg[hs].T),
            "gb2": np.ascontiguousarray(gate_b[hs] * 0.5).reshape(HPG, 1),
            "WoT": _b16(Wo[:, cs].T),
            "cw": np.ascontiguousarray(canon_w[cs, 0, :]),
            "cb": np.ascontiguousarray(canon_bias[cs]).reshape(CPG, 1),
        })
    return per_b, per_g


LAST_EXEC_NS = None
LAST_RESULTS = None


def kernel(**inputs):
    global LAST_EXEC_NS, LAST_RESULTS
    from concourse.bass_utils import run_bass_kernel_spmd

    nc = _get_program()
    per_b, per_g = host_prep(inputs)
    in_maps = []
    for core in range(8):
        b, g = divmod(core, G)
        m = {}
        m.update(per_b[b])
        m.update(per_g[g])
        in_maps.append(m)

    trace = bool(int(os.environ.get("KERNEL_TRACE", "0")))
    kw = {}
    if trace:
        tcores = os.environ.get("KERNEL_TRACE_CORES", "0")
        kw = dict(
            trace=True,
            trace_cores=[int(c) for c in tcores.split(",")],
            tmpdir=os.environ.get("KERNEL_TRACE_DIR", None),
        )
    res = run_bass_kernel_spmd(nc, in_maps, core_ids=list(range(8)), **kw)
    LAST_EXEC_NS = res.exec_time_ns
    LAST_RESULTS = res
    outp = np.zeros((B, T, C), np.float32)
    for core in range(8):
        b = core // G
        outp[b] += np.asarray(res.results[core]["out"], np.float32)
    return outp


# revision 22
# speedup vs baseline: 1.0370x; 1.0370x over previous
"""Trainium2 Bass kernel for nn_CMAModel (memory-augmented causal attention).

Sharding: 8 cores = 2 batches x 4 head-groups. Each core handles one batch and
4 heads (256 channels); the output projection is row-parallel and the 4
per-batch partials are summed on the host (bf16 partials).

Per-core device program (channels on partitions, bf16 matmul operands):
  proj: qT/kT (paired PSUM banks), V rows (with ones col for row-sums),
        gate logits -> tanh (same ACT table set as Exp; sigmoid = .5*tanh+.5)
  attention per (head-pair mq, T-chunk j): for each 128-row S-tile,
        scoresT for heads A,B as two K=64 row-tiled matmuls into a 2-bank
        PSUM pair -> ONE Exp activation over both banks -> causal tri-mask
        on diagonal tiles -> PV matmuls accumulate Ac/Am [65,512] per head
        (ones col gives row-sums Z for free)
  combine: Y = (Ac + g*Am)/Z. g and 1/Z rows live on partition 0 and are
        broadcast to 64 partitions with GpSimd partition_broadcast (no PE or
        DRAM involvement). Z-recip runs 128-wide via small SBUF reshape DMAs.
        The reciprocal-dependent final multiply is deferred past the next
        block's matmuls (software pipeline) so the in-order PE never stalls.
  conv + out-proj interleaved per T-chunk: depthwise causal conv K=4 +
        residual + bias, then out partial [512,1024] -> DRAM (bf16).
"""
import contextlib
import ctypes
import os
import sys
import types

import numpy as np

# ---------------------------------------------------------------- constants
B, T, C = 2, 2048, 1024
H, HD = 16, 64
M = 256
G = 4                 # head-groups (cores per batch)
HPG = H // G          # 4 heads per core
CPG = HPG * HD        # 256 channels per core
S = T + 2 * M         # 2560 kv rows
SM = 2 * M            # 512 memory rows
NKT = C // 128        # 8 contraction tiles
NST = S // 128        # 20 S tiles (16 chunk + 4 mem)
TC = 512              # T chunk size
NTC = T // TC         # 4
K = 4                 # conv taps
SCALE = 1.0 / float(np.sqrt(HD))

_BUILT = None


# ------------------------------------------------------- axon NTFF hook shim
def _install_ntff_hook():
    """The agent image lacks antenv.axon_hooks; synthesize it so
    run_bass_kernel_spmd(trace=True) can capture NTFF profiles."""
    if "antenv.axon_hooks" in sys.modules:
        return
    so_path = "/opt/axon/libaxon_pjrt.so"
    hook = None
    if os.path.exists(so_path):
        try:
            lib = ctypes.CDLL(so_path)
            if hasattr(lib, "axon_start_nrt_profile"):
                lib.axon_start_nrt_profile.argtypes = [
                    ctypes.POINTER(ctypes.c_int64),
                    ctypes.c_size_t,
                ]
                lib.axon_start_nrt_profile.restype = ctypes.c_int64
                lib.axon_stop_nrt_profile.argtypes = [ctypes.c_char_p]
                lib.axon_stop_nrt_profile.restype = ctypes.c_int64

                @contextlib.contextmanager
                def _hook(output_dir, device_ids):
                    import jax

                    jax.devices()
                    if device_ids:
                        ids = (ctypes.c_int64 * len(device_ids))(*device_ids)
                        rc = lib.axon_start_nrt_profile(ids, len(device_ids))
                    else:
                        rc = lib.axon_start_nrt_profile(None, 0)
                    if rc != 0:
                        raise RuntimeError(f"axon_start_nrt_profile rc={rc}")
                    try:
                        yield
                    finally:
                        n = lib.axon_stop_nrt_profile(str(output_dir).encode())
                        if n < 0:
                            raise RuntimeError(f"axon_stop_nrt_profile rc={n}")

                hook = _hook
        except OSError:
            pass
    mod = types.ModuleType("antenv.axon_hooks")
    mod.get_axon_ntff_profile_hook = lambda: hook
    mod.set_axon_ntff_profile_hook = lambda h: None
    sys.modules["antenv.axon_hooks"] = mod


# ------------------------------------------------------------- device build
def _build_program():
    import concourse.tile as tile
    from concourse import bacc, mybir
    from concourse.masks import make_upper_triangular

    f32 = mybir.dt.float32
    mdt = mybir.dt.bfloat16

    nc = bacc.Bacc("TRN2", target_bir_lowering=False, debug=False, num_devices=8)

    xT = nc.dram_tensor("xT", [C, T], mdt, kind="ExternalInput").ap()
    memT = nc.dram_tensor("memT", [C, SM], mdt, kind="ExternalInput").ap()
    WqT = nc.dram_tensor("WqT", [C, CPG], mdt, kind="ExternalInput").ap()
    WkT = nc.dram_tensor("WkT", [C, CPG], mdt, kind="ExternalInput").ap()
    WvTa = nc.dram_tensor("WvTa", [C, 64 * HPG], mdt, kind="ExternalInput").ap()
    WgT = nc.dram_tensor("WgT", [C, HPG], mdt, kind="ExternalInput").ap()
    gb2 = nc.dram_tensor("gb2", [HPG, 1], f32, kind="ExternalInput").ap()
    WoT = nc.dram_tensor("WoT", [CPG, C], mdt, kind="ExternalInput").ap()
    cw = nc.dram_tensor("cw", [CPG, K], f32, kind="ExternalInput").ap()
    cb = nc.dram_tensor("cb", [CPG, 1], f32, kind="ExternalInput").ap()
    out = nc.dram_tensor("out", [T, C], mdt, kind="ExternalOutput").ap()

    Exp = mybir.ActivationFunctionType.Exp
    Tanh = mybir.ActivationFunctionType.Tanh

    with tile.TileContext(nc) as tc:
        with contextlib.ExitStack() as ctx:
            const = ctx.enter_context(tc.tile_pool(name="const", bufs=1))
            xpool = ctx.enter_context(tc.tile_pool(name="xpool", bufs=2))
            sb = ctx.enter_context(tc.tile_pool(name="sb", bufs=1))
            work = ctx.enter_context(tc.tile_pool(name="work", bufs=4))
            small = ctx.enter_context(tc.tile_pool(name="small", bufs=1))
            psum = ctx.enter_context(
                tc.tile_pool(name="psum", bufs=1, space="PSUM")
            )

            # ---- constants / weights. Order matters: the first proj
            # matmul needs only wq/wk + the first x k-tiles, so issue those
            # DMAs first and defer the rest.
            wq_s = const.tile([128, NKT, CPG], mdt)
            nc.sync.dma_start(out=wq_s, in_=WqT.rearrange("(a p) n -> p a n", p=128))
            xTr0 = xT.rearrange("(a p) t -> p a t", p=128)
            xh0 = xpool.tile([128, NKT, T // 2], mdt, tag="xbig", name="xh0")
            nc.sync.dma_start(out=xh0[:, 0, :], in_=xTr0[:, 0, :T // 2])
            wk_s = const.tile([128, NKT, CPG], mdt)
            nc.sync.dma_start(out=wk_s, in_=WkT.rearrange("(a p) n -> p a n", p=128))
            for k in range(1, NKT):
                nc.sync.dma_start(out=xh0[:, k, :], in_=xTr0[:, k, :T // 2])
            wva_s = const.tile([128, NKT, 64 * HPG], mdt)
            nc.sync.dma_start(out=wva_s, in_=WvTa.rearrange("(a p) n -> p a n", p=128))
            wg_s = const.tile([128, NKT, HPG], mdt)
            nc.sync.dma_start(out=wg_s, in_=WgT.rearrange("(a p) n -> p a n", p=128))
            mems = xpool.tile([128, NKT, SM], mdt, tag="xmem", name="xmem")
            nc.sync.dma_start(out=mems, in_=memT.rearrange("(a p) t -> p a t", p=128))
            wo_s = const.tile([128, 2, C], mdt)
            nc.sync.dma_start(out=wo_s, in_=WoT.rearrange("(a p) n -> p a n", p=128))
            cw_s = const.tile([128, 2, K], f32)
            nc.sync.dma_start(out=cw_s, in_=cw.rearrange("(a p) n -> p a n", p=128))
            cb_s = const.tile([128, 2, 1], f32)
            nc.sync.dma_start(out=cb_s, in_=cb.rearrange("(a p) n -> p a n", p=128))
            gb2_s = const.tile([HPG, 1], f32)
            nc.sync.dma_start(out=gb2_s, in_=gb2)

            trif = const.tile([128, 128], f32)
            make_upper_triangular(nc, trif, val=1.0, diag=True)
            tri2 = const.tile([128, 2, 128], mdt)
            nc.vector.tensor_copy(tri2[:, 0, :], trif)
            nc.vector.tensor_copy(tri2[:, 1, :], trif)

            # ---- persistent activations
            # qkT_s[:, m, 0, t] = qT, [:, m, 1, t] = kT  (m = channel half)
            qkT_s = sb.tile([128, 2, 2, T], mdt)
            kTm_s = sb.tile([128, 2, SM], mdt)
            V_s = sb.tile([128, NST, 128 * HPG], mdt)
            # broadcast sources all live on partition 0 (engine ops need
            # 32-aligned partition starts): slot 4*mq+{0,1,2,3} =
            # {sigmoid_A, sigmoid_B, recipZ_A, recipZ_B}
            gzt = sb.tile([1, 8, T], mdt)
            # attnout[:, mq, 0, t] = Y, [:, mq, 1, t] = conv result
            attnout = sb.tile([128, 2, 2, T], mdt)

            # one-time inits: ones col in V, gzt ones + recip rows (rows 0-1 /
            # 64-65 are overwritten by the gate tanh per chunk; engine ops
            # need 32-aligned partition starts so memset the whole block)
            oc = V_s[:, :, 64:128 * HPG:128]
            nc.vector.memset(oc, 1.0)

            xTr = xT.rearrange("(a p) t -> p a t", p=128)

            def proj_chunk(xh, tglob, tloc, on_act):
                """q/k/V/gate projections for T columns [tglob, tglob+512).
                on_act: route PSUM->SBUF copies to ScalarE (idle early) or
                VectorE (when ScalarE is busy with attention exps)."""
                cp = nc.scalar.copy if on_act else nc.vector.tensor_copy
                for m in range(2):
                    for w, ws in ((0, wq_s), (1, wk_s)):
                        qk = psum.tile([128, TC], f32, tag="pp", bufs=2,
                                       name=f"qk{tglob}_{m}_{w}")
                        for k in range(NKT):
                            nc.tensor.matmul(
                                qk,
                                ws[:, k, m * 128:(m + 1) * 128],
                                xh[:, k, tloc:tloc + TC],
                                start=(k == 0),
                                stop=(k == NKT - 1),
                            )
                        cp(qkT_s[:, m, w, tglob:tglob + TC], qk)
                for mt in range(TC // 128):
                    st = tglob // 128 + mt
                    pv = psum.tile([128, 64 * HPG], f32, tag="pa", bufs=2,
                                   name=f"pv{st}")
                    for k in range(NKT):
                        nc.tensor.matmul(
                            pv,
                            xh[:, k, tloc + mt * 128:tloc + (mt + 1) * 128],
                            wva_s[:, k, :],
                            start=(k == 0),
                            stop=(k == NKT - 1),
                        )
                    # copy the 4 x 64 v-blocks, skipping the ones columns
                    cp(
                        V_s[:, st, :].rearrange("p (h c) -> p h c", c=128)[:, :, 0:64],
                        pv.rearrange("p (h c) -> p h c", c=64),
                    )
                pg = psum.tile([HPG, TC], f32, tag="pa", bufs=2,
                               name=f"pg{tglob}")
                for k in range(NKT):
                    nc.tensor.matmul(
                        pg,
                        wg_s[:, k, :],
                        xh[:, k, tloc:tloc + TC],
                        start=(k == 0),
                        stop=(k == NKT - 1),
                    )
                # sigmoid(l+b) = .5*tanh((l+b)/2) + .5; the .5 affine folds
                # into the gbc broadcast matmul via the ind/ones-row coeffs
                gtmp = small.tile([HPG, TC], mdt, tag="gt", bufs=2,
                                  name=f"gt{tglob}")
                nc.scalar.activation(
                    gtmp, pg, Tanh, bias=gb2_s, scale=0.5,
                )
                nc.vector.tensor_scalar(
                    gtmp, gtmp, 0.5, 0.5,
                    mybir.AluOpType.mult, mybir.AluOpType.add,
                )
                nc.sync.dma_start(
                    out=gzt[0:1, 0:2, tglob:tglob + TC], in_=gtmp[0:2, :]
                )
                nc.sync.dma_start(
                    out=gzt[0:1, 4:6, tglob:tglob + TC], in_=gtmp[2:4, :]
                )

            def proj_mem(mems):
                mk = psum.tile([128, 2, SM], f32, tag="pa", bufs=2, name="mk")
                for m in range(2):
                    for k in range(NKT):
                        nc.tensor.matmul(
                            mk[:, m, :],
                            wk_s[:, k, m * 128:(m + 1) * 128],
                            mems[:, k, :],
                            start=(k == 0),
                            stop=(k == NKT - 1),
                        )
                nc.scalar.copy(kTm_s, mk)
                for mt in range(SM // 128):
                    st = 16 + mt
                    pv = psum.tile([128, 64 * HPG], f32, tag="pa", bufs=2,
                                   name=f"pvm{mt}")
                    for k in range(NKT):
                        nc.tensor.matmul(
                            pv,
                            mems[:, k, mt * 128:(mt + 1) * 128],
                            wva_s[:, k, :],
                            start=(k == 0),
                            stop=(k == NKT - 1),
                        )
                    nc.scalar.copy(
                        V_s[:, st, :].rearrange("p (h c) -> p h c", c=128)[:, :, 0:64],
                        pv.rearrange("p (h c) -> p h c", c=64),
                    )

            def attn_block(mq, j):
                """Attention accumulation for head pair (2mq, 2mq+1), chunk j.
                Emits everything up to uY = Ac + g*Am (which frees the PSUM
                accumulators without waiting for the Z-reciprocal chain) and
                returns a finish() closure — the reciprocal-dependent final
                multiply — to be emitted after the NEXT block's matmuls so
                the in-order PE queue never stalls on the Z chain."""
                sl = 4 * mq
                hA, hB = 2 * mq, 2 * mq + 1
                nct = 4 * (j + 1)
                js = TC * j
                # gate broadcast on GpSimd (idle engine, off the PE queue)
                gbS = small.tile([64, 2, TC], mdt, tag="gbS", bufs=3,
                                 name=f"gb{mq}_{j}")
                for hb in range(2):
                    nc.gpsimd.partition_broadcast(
                        gbS[:, hb, :], gzt[0:1, sl + hb, js:js + TC]
                    )
                AcAm_A = psum.tile([128, 2, TC], f32, tag="pa", bufs=2,
                                   name=f"aa{mq}_{j}")
                AcAm_B = psum.tile([128, 2, TC], f32, tag="pa", bufs=2,
                                   name=f"ab{mq}_{j}")
                for i in range(nct + 4):
                    is_mem = i >= nct
                    si = (16 + i - nct) if is_mem else i
                    off = 0
                    if not is_mem and si >= 4 * j:
                        off = 128 * si - TC * j
                    sp = psum.tile([128, 2, TC], f32, tag="pp", bufs=2,
                                   name=f"sp{mq}_{j}_{i}")
                    for b, ro in ((0, 0), (1, 64)):
                        kt = (
                            qkT_s[ro:ro + 64, mq, 1, si * 128:(si + 1) * 128]
                            if si < 16
                            else kTm_s[ro:ro + 64, mq,
                                       (si - 16) * 128:(si - 15) * 128]
                        )
                        nc.tensor.matmul(
                            sp[:, b, off:],
                            kt,
                            qkT_s[ro:ro + 64, mq, 0, js + off:js + TC],
                            start=True,
                            stop=True,
                        )
                    Pt = work.tile([128, 2, TC], mdt, tag="P", bufs=6)
                    nc.scalar.activation(
                        Pt[:, :, off:], sp[:, :, off:], Exp, scale=SCALE
                    )
                    if not is_mem and si >= 4 * j:
                        nc.vector.tensor_mul(
                            Pt[:, :, off:off + 128], Pt[:, :, off:off + 128],
                            tri2,
                        )
                    cm = 1 if is_mem else 0
                    first = (i == 0) or (i == nct)
                    last = (i == nct - 1) or (i == nct + 3)
                    nc.tensor.matmul(
                        AcAm_A[:, cm, off:],
                        V_s[:, si, 128 * hA:128 * hA + 128],
                        Pt[:, 0, off:],
                        start=first,
                        stop=last,
                    )
                    nc.tensor.matmul(
                        AcAm_B[:, cm, off:],
                        V_s[:, si, 128 * hB:128 * hB + 128],
                        Pt[:, 1, off:],
                        start=first,
                        stop=last,
                    )
                # Z rows -> 128-wide reciprocal -> gzt recip rows (TT may
                # read at most one PSUM operand, so copy then add)
                zt = small.tile([128, 2, TC], f32, tag="zt", bufs=2,
                                name=f"zt{mq}_{j}")
                uYs = []
                for b, AcAm in ((0, AcAm_A), (1, AcAm_B)):
                    nc.vector.tensor_copy(zt[64:65, b, :], AcAm[64:65, 0, :])
                    nc.vector.tensor_add(
                        zt[64:65, b, :], zt[64:65, b, :], AcAm[64:65, 1, :]
                    )
                    # uY = Ac + g*Am consumes the accumulators now
                    uY = small.tile([64, TC], mdt, tag="uY", bufs=5,
                                    name=f"uY{mq}_{j}_{b}")
                    nc.vector.tensor_mul(uY, AcAm[0:64, 1, :], gbS[:, b, :])
                    nc.vector.tensor_add(uY, uY, AcAm[0:64, 0, :])
                    uYs.append(uY)
                zrf = small.tile([128, 8], f32, tag="zrf", bufs=2,
                                 name=f"zrf{mq}_{j}")
                nc.sync.dma_start(out=zrf, in_=zt[64:65, :, :])
                zrg = small.tile([128, 8], f32, tag="zrg", bufs=2,
                                 name=f"zrg{mq}_{j}")
                nc.vector.reciprocal(zrg, zrf)
                zrb = small.tile([128, 8], mdt, tag="zrb", bufs=2,
                                 name=f"zrb{mq}_{j}")
                nc.vector.tensor_copy(zrb, zrg)
                nc.sync.dma_start(
                    out=gzt[0:1, sl + 2, js:js + TC], in_=zrb[0:64, :]
                )
                nc.sync.dma_start(
                    out=gzt[0:1, sl + 3, js:js + TC], in_=zrb[64:128, :]
                )

                rbS = small.tile([64, 2, TC], mdt, tag="rbS", bufs=3,
                                 name=f"rb{mq}_{j}")
                for hb in range(2):
                    nc.gpsimd.partition_broadcast(
                        rbS[:, hb, :], gzt[0:1, sl + 2 + hb, js:js + TC]
                    )

                def finish():
                    nc.vector.tensor_mul(
                        attnout[0:64, mq, 0, js:js + TC], uYs[0], rbS[:, 0, :]
                    )
                    ybt = small.tile([64, TC], mdt, tag="ybt", bufs=2,
                                     name=f"yb{mq}_{j}")
                    nc.vector.tensor_mul(ybt, uYs[1], rbS[:, 1, :])
                    nc.sync.dma_start(
                        out=attnout[64:128, mq, 0, js:js + TC], in_=ybt
                    )

                return finish

            def conv_chunk(j, mq):
                """depthwise causal conv + residual + bias on chunk j."""
                eng = nc.vector
                js, je = TC * j, TC * (j + 1)
                y = attnout[:, mq, 0, :]
                R = attnout[:, mq, 1, :]
                eng.tensor_scalar_add(
                    R[:, js:je], y[:, js:je], cb_s[:, mq, :]
                )
                ctmp = small.tile([128, TC], mdt, tag=f"ctmp{mq}", bufs=2,
                                  name=f"ct{j}_{mq}")
                for k in range(K):
                    sh = K - 1 - k
                    if sh == 0:
                        eng.tensor_scalar_mul(
                            ctmp, y[:, js:je], cw_s[:, mq, k:k + 1]
                        )
                        eng.tensor_add(R[:, js:je], R[:, js:je], ctmp)
                    else:
                        a = sh if j == 0 else 0
                        eng.tensor_scalar_mul(
                            ctmp[:, a:], y[:, js + a - sh:je - sh],
                            cw_s[:, mq, k:k + 1],
                        )
                        eng.tensor_add(
                            R[:, js + a:je], R[:, js + a:je], ctmp[:, a:]
                        )

            def outproj_chunk(j, mts=None):
                for mt in (range(TC // 128) if mts is None else mts):
                    row = j * 4 + mt
                    po = psum.tile([128, 2, TC], f32, tag="pa", bufs=2,
                                   name=f"po{row}")
                    for nb in range(2):
                        for p in range(2):
                            nc.tensor.matmul(
                                po[:, nb, :],
                                attnout[:, p, 1, row * 128:(row + 1) * 128],
                                wo_s[:, p, nb * TC:(nb + 1) * TC],
                                start=(p == 0),
                                stop=(p == 1),
                            )
                    ot = small.tile([128, 2, TC], mdt, tag="ot", bufs=3,
                                    name=f"ot{row}")
                    if mt % 2 == 0:
                        nc.vector.tensor_copy(ot, po)
                    else:
                        nc.scalar.copy(ot, po)
                    nc.sync.dma_start(
                        out=out[row * 128:(row + 1) * 128, :].rearrange(
                            "p (a n) -> p a n", a=2
                        ),
                        in_=ot,
                    )

            # ---- emission: proj c0, mem, c1, then attn j interleaved with
            # remaining proj chunks so PE always has dense work and ACT/DVE
            # overlap.
            proj_chunk(xh0, 0, 0, on_act=True)
            proj_mem(mems)

            xh1 = xpool.tile([128, NKT, T // 2], mdt, tag="xbig", name="xh1")
            for k in range(NKT):
                nc.sync.dma_start(out=xh1[:, k, :], in_=xTr[:, k, T // 2:])

            # Pipelined emission: each block's reciprocal-dependent finish()
            # lands after the next block's matmul burst; conv one slot later;
            # outproj one more. Keeps the in-order PE queue stall-free.
            pending = []

            def drain(now):
                pending.sort(key=lambda e: e[0])
                while pending and pending[0][0] <= now:
                    pending.pop(0)[1]()

            slot = 0
            for j in range(NTC):
                for mq in range(2):
                    fin = attn_block(mq, j)
                    drain(slot)

                    def fin_conv(f=fin, jc=j, mqc=mq):
                        f()
                        conv_chunk(jc, mqc)

                    pending.append((slot + 1, fin_conv))
                    slot += 1
                if j < NTC - 1:
                    tg = (j + 1) * TC
                    if tg < T // 2:
                        proj_chunk(xh0, tg, tg, on_act=True)
                    else:
                        proj_chunk(xh1, tg, tg - T // 2, on_act=False)
                    drain(slot)
                    slot += 1
                pending.append((slot + 1, lambda jc=j: outproj_chunk(jc)))
            drain(slot + 3)

    nc.compile()
    return nc


def _get_program():
    global _BUILT
    if _BUILT is None:
        _install_ntff_hook()
        _BUILT = _build_program()
    return _BUILT


# --------------------------------------------------------------- host side
def _b16(a):
    import ml_dtypes

    return np.ascontiguousarray(a, np.float32).astype(ml_dtypes.bfloat16)


def host_prep(inputs):
    x = np.ascontiguousarray(np.asarray(inputs["x"], np.float32))
    fwd = np.asarray(inputs["fwd_mem"], np.float32)
    rev = np.asarray(inputs["rev_mem"], np.float32)
    Wq = np.asarray(inputs["Wq"], np.float32)
    Wk = np.asarray(inputs["Wk"], np.float32)
    Wv = np.asarray(inputs["Wv"], np.float32)
    Wo = np.asarray(inputs["Wo"], np.float32)
    gate_w = np.asarray(inputs["gate_w"], np.float32)
    gate_b = np.asarray(inputs["gate_b"], np.float32)
    canon_w = np.asarray(inputs["canon_w"], np.float32)
    canon_bias = np.asarray(inputs["canon_bias"], np.float32)

    Wg = (gate_w.astype(np.float64) @ Wq.astype(np.float64)).astype(np.float32)

    per_b, per_g = [], []
    for b in range(B):
        per_b.append({
            "xT": _b16(x[b].T),
            "memT": _b16(np.concatenate([fwd[b], rev[b]], axis=0).T),
        })
    for g in range(G):
        cs = slice(g * CPG, (g + 1) * CPG)
        WvTa = np.ascontiguousarray(Wv[cs].T)
        hs = slice(g * HPG, (g + 1) * HPG)
        per_g.append({
            "WqT": _b16(Wq[cs].T),
            "WkT": _b16(Wk[cs].T),
            "WvTa": _b16(WvTa),
            "WgT": _b16(Wg[hs].T),
            "gb2": np.ascontiguousarray(gate_b[hs] * 0.5).reshape(HPG, 1),
            "WoT": _b16(Wo[:, cs].T),
            "cw": np.ascontiguousarray(canon_w[cs, 0, :]),
            "cb": np.ascontiguousarray(canon_bias[cs]).reshape(CPG, 1),
        })
    return per_b, per_g


LAST_EXEC_NS = None
LAST_RESULTS = None


def kernel(**inputs):
    global LAST_EXEC_NS, LAST_RESULTS
    from concourse.bass_utils import run_bass_kernel_spmd

    nc = _get_program()
    per_b, per_g = host_prep(inputs)
    in_maps = []
    for core in range(8):
        b, g = divmod(core, G)
        m = {}
        m.update(per_b[b])
        m.update(per_g[g])
        in_maps.append(m)

    trace = bool(int(os.environ.get("KERNEL_TRACE", "0")))
    kw = {}
    if trace:
        tcores = os.environ.get("KERNEL_TRACE_CORES", "0")
        kw = dict(
            trace=True,
            trace_cores=[int(c) for c in tcores.split(",")],
            tmpdir=os.environ.get("KERNEL_TRACE_DIR", None),
        )
    res = run_bass_kernel_spmd(nc, in_maps, core_ids=list(range(8)), **kw)
    LAST_EXEC_NS = res.exec_time_ns
    LAST_RESULTS = res
    outp = np.zeros((B, T, C), np.float32)
    for core in range(8):
        b = core // G
        outp[b] += np.asarray(res.results[core]["out"], np.float32)
    return outp
